# revision 3
# baseline (speedup 1.0000x reference)
"""Trainium2 Bass kernel for pairwise IoU (nms_detection).

Computes, for dt boxes D[2000,4] and gt boxes G[512,4]:
  dt_gt_iou [2000,512] f32, dt_dt_iou [2000,2000] f32,
  neighbour_mask = dt_dt_iou >= 0.2 (bool).

Sharding: rows (dt boxes) split across 8 cores (250 rows each); each core
computes its row-band against a unified column list [dt | gt] of 2512 boxes.

Per-core device pipeline (column chunks of 512, rows in partition tiles of
128+122):
  PE      : broadcast column-box fields (x1,y1,x2,y2,area) into PSUM planes
  ACT     : copy bx1/by1/barea planes PSUM->SBUF (bx2/by2 stay in PSUM)
  DVE     : wr = relu(min(bx2,ax2)-max(bx1,ax1))   [custom fused op]
            hr = likewise for y                     [custom fused op]
            inter = wr*hr
            union = (barea+aarea)-inter             [custom fused op]
            r = reciprocal_approx_fast(union)
            iou = inter*r
            mask = iou >= 0.2 (u8)
  DMA out : per-chunk HWDGE
"""

import os

import numpy as np

N_DT = 2000
N_GT = 512
N_CORES = 8
ROWS = N_DT // N_CORES  # 250
W = N_DT + N_GT  # unified column count
IOU_THR = 0.2
CHUNK = 512

PROFILE = os.environ.get("KERNEL_PROFILE", "0") == "1"
LAST_EXEC_NS = None
LAST_RESULTS = None

_OPS = None
_PROG = None


def _register_ops():
    """Register the fused custom DVE ops (idempotent)."""
    global _OPS
    if _OPS is not None:
        return _OPS
    from concourse import dve_ops
    from concourse.dve_spec import Spec, Src0, Src1, C0, C1, lower, maxx, minn, relu
    from concourse.dve_uop import DveOpSpec

    def make(name, spec, rd1_en=True):
        if name in dve_ops._SUB_OPCODE_FOR_NAME:
            for op in dve_ops.OPS:
                if op.name == name:
                    return op
        row = 1 + len(dve_ops.OPS)
        shas = {}
        for ver in ("v3", "v4"):
            shas[ver] = DveOpSpec(
                name=name, opcode=row, uops=lower(spec, ver=ver), rd1_en=rd1_en
            ).sha(ver)
        op = dve_ops.DveOp(name, spec, subdim=False, uops_sha=shas)
        dve_ops.OPS.append(op)
        dve_ops.CUSTOM_DVE_SPECS[name] = spec
        dve_ops._SUB_OPCODE_FOR_NAME[name] = row
        return op

    # out = relu(min(in0, s0) - max(in1, s1)); in0=bx2(psum), in1=bx1(sbuf)
    overlap = make(
        "IOU_OVERLAP_ANT",
        Spec(
            body=relu(minn(Src0, C0) - maxx(Src1, C1)),
            reference=lambda in0, in1, s0, s1, imm2: np.maximum(
                np.minimum(in0, s0) - np.maximum(in1, s1), 0.0
            ),
        ),
    )
    # out = (in0 + s0) - in1; in0=barea, in1=inter, s0=aarea
    union = make(
        "IOU_UNION_ANT",
        Spec(
            body=(Src0 + C0) - Src1,
            reference=lambda in0, in1, s0, s1, imm2: (in0 + s0) - in1,
        ),
    )
    _OPS = (overlap, union)
    return _OPS


def _build_program():
    global _PROG
    if _PROG is not None:
        return _PROG
    import concourse.bacc as bacc
    import concourse.tile as tile
    from concourse import mybir

    overlap_op, union_op = _register_ops()

    f32 = mybir.dt.float32
    u8 = mybir.dt.uint8
    Alu = mybir.AluOpType

    nc = bacc.Bacc("TRN2", target_bir_lowering=False, debug=False)

    a_dram = nc.dram_tensor("a_slab", [ROWS, 5], f32, kind="ExternalInput").ap()
    bT_dram = nc.dram_tensor("bT_slab", [5, W], f32, kind="ExternalInput").ap()
    iou_dram = nc.dram_tensor("iou_out", [ROWS, W], f32, kind="ExternalOutput").ap()
    mask_dram = nc.dram_tensor("mask_out", [ROWS, W], u8, kind="ExternalOutput").ap()

    n_chunks = (W + CHUNK - 1) // CHUNK

    with tile.TileContext(nc) as tc:
        import contextlib

        with contextlib.ExitStack() as ctx:
            constp = ctx.enter_context(tc.tile_pool(name="const", bufs=1))
            bcastp = ctx.enter_context(tc.tile_pool(name="bcast", bufs=1))
            psum_keep = ctx.enter_context(
                tc.tile_pool(name="psum_keep", bufs=2, space="PSUM")
            )
            psum_tmp = ctx.enter_context(
                tc.tile_pool(name="psum_tmp", bufs=2, space="PSUM")
            )
            workp = ctx.enter_context(tc.tile_pool(name="work", bufs=3))
            outp = ctx.enter_context(tc.tile_pool(name="outp", bufs=3))

            # --- load per-core row slab (a side) and column rows (b side) ---
            a0 = constp.tile([128, 5], f32)
            nc.sync.dma_start(a0[:], a_dram[0:128, :])
            a1 = constp.tile([ROWS - 128, 5], f32)
            nc.sync.dma_start(a1[:], a_dram[128:ROWS, :])

            brow = []  # five [1, W] tiles: x1,y1,x2,y2,area
            for f in range(5):
                t = constp.tile([1, W], f32, tag=f"brow{f}")
                nc.sync.dma_start(t[:], bT_dram[f : f + 1, :])
                brow.append(t)

            ones = constp.tile([1, 128], f32)
            nc.vector.memset(ones[:], 1.0)

            # full-width SBUF broadcast tensors for bx1/by1/barea
            bx1s = bcastp.tile([128, W], f32)
            by1s = bcastp.tile([128, W], f32)
            bareas = bcastp.tile([128, W], f32)

            rts = [(a0, 128, 0), (a1, ROWS - 128, 128)]

            for c in range(n_chunks):
                lo = c * CHUNK
                L = min(W - lo, CHUNK)
                cs = slice(lo, lo + L)

                # PE broadcasts of the 5 column fields
                p_bx2 = psum_keep.tile([128, CHUNK], f32, tag="p_bx2")
                nc.tensor.matmul(p_bx2[:, :L], ones[:], brow[2][:, cs])
                p_by2 = psum_keep.tile([128, CHUNK], f32, tag="p_by2")
                nc.tensor.matmul(p_by2[:, :L], ones[:], brow[3][:, cs])

                for f, dst in ((0, bx1s), (1, by1s), (4, bareas)):
                    p_t = psum_tmp.tile([128, CHUNK], f32, tag="p_tmp")
                    nc.tensor.matmul(p_t[:, :L], ones[:], brow[f][:, cs])
                    nc.scalar.copy(dst[:, cs], p_t[:, :L])

                for a, P, r0 in rts:
                    ax1 = a[:, 0:1]
                    ay1 = a[:, 1:2]
                    ax2 = a[:, 2:3]
                    ay2 = a[:, 3:4]
                    aarea = a[:, 4:5]

                    wr = workp.tile([128, CHUNK], f32, tag="wr")
                    nc.vector._custom_dve(
                        overlap_op,
                        out=wr[0:P, :L],
                        in0=p_bx2[0:P, :L],
                        in1=bx1s[0:P, cs],
                        s0=ax2,
                        s1=ax1,
                    )
                    hr = workp.tile([128, CHUNK], f32, tag="hr")
                    nc.vector._custom_dve(
                        overlap_op,
                        out=hr[0:P, :L],
                        in0=p_by2[0:P, :L],
                        in1=by1s[0:P, cs],
                        s0=ay2,
                        s1=ay1,
                    )
                    inter = workp.tile([128, CHUNK], f32, tag="inter")
                    nc.vector.tensor_mul(inter[0:P, :L], wr[0:P, :L], hr[0:P, :L])

                    un = workp.tile([128, CHUNK], f32, tag="un")
                    nc.vector._custom_dve(
                        union_op,
                        out=un[0:P, :L],
                        in0=bareas[0:P, cs],
                        in1=inter[0:P, :L],
                        s0=aarea,
                    )
                    rec = workp.tile([128, CHUNK], f32, tag="rec")
                    nc.vector.reciprocal_approx_fast(out=rec[0:P, :L], in_=un[0:P, :L])

                    iou_t = outp.tile([128, CHUNK], f32, tag="iou_t")
                    nc.vector.tensor_mul(iou_t[0:P, :L], inter[0:P, :L], rec[0:P, :L])
                    mask_t = outp.tile([128, CHUNK], u8, tag="mask_t")
                    nc.vector.tensor_scalar(
                        mask_t[0:P, :L], iou_t[0:P, :L], IOU_THR, None, Alu.is_ge
                    )

                    nc.sync.dma_start(iou_dram[r0 : r0 + P, cs], iou_t[0:P, :L])
                    nc.sync.dma_start(mask_dram[r0 : r0 + P, cs], mask_t[0:P, :L])

    nc.compile()
    _PROG = nc
    return nc


def _box5(boxes):
    x1 = boxes[:, 0]
    y1 = boxes[:, 1]
    x2 = boxes[:, 2]
    y2 = boxes[:, 3]
    area = (x2 - x1) * (y2 - y1)
    return np.stack([x1, y1, x2, y2, area], axis=1).astype(np.float32)


def kernel(detections, gt_boxes):
    global LAST_EXEC_NS, LAST_RESULTS
    detections = np.asarray(detections, dtype=np.float32)[:N_DT]
    gt_boxes = np.asarray(gt_boxes, dtype=np.float32)

    dt5 = _box5(detections)  # [2000, 5]
    gt5 = _box5(gt_boxes)  # [512, 5]
    bT = np.concatenate([dt5, gt5], axis=0).T.copy()  # [5, 2512]

    nc = _build_program()

    in_maps = []
    for c in range(N_CORES):
        in_maps.append(
            {
                "a_slab": np.ascontiguousarray(dt5[c * ROWS : (c + 1) * ROWS]),
                "bT_slab": bT,
            }
        )

    from concourse.bass_utils import run_bass_kernel_spmd

    kwargs = {}
    if PROFILE:
        import concourse.bass_utils as _bu

        _bu.upload_artifacts = lambda d: d  # no S3 in this container
        kwargs = dict(trace=True, trace_cores=[0])
    res = run_bass_kernel_spmd(nc, in_maps, list(range(N_CORES)), **kwargs)
    LAST_RESULTS = res
    LAST_EXEC_NS = res.exec_time_ns

    iou = np.concatenate([res.results[c]["iou_out"] for c in range(N_CORES)], axis=0)
    mask = np.concatenate([res.results[c]["mask_out"] for c in range(N_CORES)], axis=0)

    dt_dt_iou = np.ascontiguousarray(iou[:, :N_DT])
    dt_gt_iou = np.ascontiguousarray(iou[:, N_DT:])
    neighbour_mask = mask[:, :N_DT] != 0

    return dt_gt_iou, dt_dt_iou, neighbour_mask
